# revision 1
# baseline (speedup 1.0000x reference)
"""Causal self-attention (B=2, T=2048, D=1024, H=16, Dh=64) on 8 NeuronCores.

Sharding: tensor-parallel over heads. Core c owns heads {2c, 2c+1}:
  - QKV: computes q/k/v columns c*128:(c+1)*128 of each section.
      q,k are produced transposed (qT/kT: [128 qkv-cols, tokens]) via
      out = w3_slice.T @ x.T matmuls; v is produced in natural layout
      ([tokens, 128 v-cols]) via out = x @ w3_vslice matmuls.
  - Attention: for each (batch, q-chunk of 512, k-tile of 128):
      S^T = K_h @ Q_h.T directly from kT/qT (both heads packed in the
      128x128 PE array via disjoint 64-row groups), exp on ACT (no max
      subtraction needed: |S*scale| <= ~6), causal mask via affine_select
      on the diagonal tiles (fill=0 after exp), then out^T accumulated as
      V'.T @ P^T where V' = [V | ones]: row 64 of the PSUM accumulator is
      the softmax denominator. Normalization multiplies by a reciprocal
      row broadcast across partitions with a K=1 matmul.
  - Projection: partial out^T = w_proj_slice.T applied per 128-row slice;
      per-core partial [1024, 4096] outputs are summed on the host.

All matmuls run in float32r (4-byte data, reduced-precision multiply,
1 cycle/row for moving dims >= 256 -- 4x faster than plain fp32).

TRN2 allows at most one sync-wait per instruction; bacc's
generate_event_semaphores pass splits multi-wait instructions, which is
why the program is built with bacc.Bacc and compiled before dispatch.
The causal mask is applied out-of-place (pt -> pt2) so diagonal PV
matmuls depend only on the GPSIMD masker; the softmax normalize chain
(exact reciprocal + DRAM-bounce partition broadcast + multiply) runs
off the PE-critical path after PV accumulators are copied to SBUF.
"""

import numpy as np

D_MODEL = 1024
B, T = 2, 2048
RC = 128  # per-core qkv columns per q/k/v section == per-core w_proj rows
M = B * T
N_CORES = 8

_prog_cache = {}
_last_results = None  # BassKernelResults of the most recent run (for profiling)


def build_program(Tb=T, use_vbias=False):
    from contextlib import ExitStack

    import concourse.bass as bass
    import concourse.tile as tile
    from concourse import bacc, mybir
    from concourse.tile import add_dep_helper

    f32 = mybir.dt.float32
    f32r = mybir.dt.float32r
    EXP = mybir.ActivationFunctionType.Exp
    MULT = mybir.AluOpType.mult
    IS_GE = mybir.AluOpType.is_ge

    Mb = B * Tb
    mc_per_b = Tb // 512  # x/m chunks of 512 tokens per batch
    mt_per_b = Tb // 128  # v tiles of 128 tokens per batch
    n_qc = Tb // 512      # query chunks per batch

    nc = bacc.Bacc("TRN2", target_bir_lowering=False, debug=False)
    xT = nc.dram_tensor("xT", [D_MODEL, Mb], f32r, kind="ExternalInput").ap()
    w3 = nc.dram_tensor("w3", [D_MODEL, 3 * RC], f32r, kind="ExternalInput").ap()
    wp = nc.dram_tensor("wp", [RC, D_MODEL], f32r, kind="ExternalInput").ap()
    bqk = nc.dram_tensor("bqk", [RC, 2], f32, kind="ExternalInput").ap()
    ident = nc.dram_tensor("ident", [128, 128], f32r, kind="ExternalInput").ap()
    bv = None
    if use_vbias:
        bv = nc.dram_tensor("bv", [RC, 1], f32, kind="ExternalInput").ap()
    out_d = nc.dram_tensor("out", [D_MODEL, Mb], f32, kind="ExternalOutput").ap()
    scr_d = nc.dram_tensor("scr", [2 * n_qc, 2, 512], f32).ap()  # recip bounce

    xT_r = xT.rearrange("(kt p) m -> p kt m", p=128)  # [128, 8, Mb]
    w3_r = w3.rearrange("(kt p) n -> p kt n", p=128)  # [128, 8, 384]

    with tile.TileContext(nc) as tc:
        with ExitStack() as ctx:
            singles = ctx.enter_context(tc.tile_pool(name="singles", bufs=1))
            xpool = ctx.enter_context(tc.tile_pool(name="xpool", bufs=3))
            ptp = ctx.enter_context(tc.tile_pool(name="ptp", bufs=3))
            pt2p = ctx.enter_context(tc.tile_pool(name="pt2p", bufs=2))
            rcp = ctx.enter_context(tc.tile_pool(name="rcp", bufs=2))
            rbp = ctx.enter_context(tc.tile_pool(name="rbp", bufs=2))
            vtp = ctx.enter_context(tc.tile_pool(name="vtp", bufs=2))
            pvcp = ctx.enter_context(tc.tile_pool(name="pvcp", bufs=3))
            obp = ctx.enter_context(tc.tile_pool(name="obp", bufs=3))
            ps_a = ctx.enter_context(tc.tile_pool(name="ps_a", bufs=2, space="PSUM"))
            ps_s = ctx.enter_context(tc.tile_pool(name="ps_s", bufs=1, space="PSUM"))
            ps_pv = ctx.enter_context(tc.tile_pool(name="ps_pv", bufs=2, space="PSUM"))

            # identity first (tiny), then PE warmup matmuls so the HAM clock
            # gate is released by the time the first x chunk lands
            id_sb = singles.tile([128, 128], f32r, tag="ident")
            nc.sync.dma_start(id_sb, ident)
            wu_ps = ps_a.tile([128, 512], f32, tag="mm")
            for _ in range(56):
                nc.tensor.matmul(wu_ps[:, 0:128], id_sb, id_sb,
                                 start=True, stop=True)

            # x chunks prefetched up front on the SP HWDGE ring; weights go
            # through the ACT HWDGE ring so the two streams don't serialize
            x_tiles = []
            for mc in range(B * mc_per_b):
                x_sb = xpool.tile([128, 8, 512], f32r, tag="x")
                nc.sync.dma_start(x_sb, xT_r[:, :, mc * 512:(mc + 1) * 512])
                x_tiles.append(x_sb)

            w3_sb = singles.tile([128, 8, 3 * RC], f32r, tag="w3")
            nc.scalar.dma_start(w3_sb, w3_r)
            wp_sb = singles.tile([128, D_MODEL], f32r, tag="wp")
            nc.scalar.dma_start(wp_sb, wp)
            bqk_sb = singles.tile([RC, 2], f32, tag="bqk")
            nc.scalar.dma_start(bqk_sb, bqk)
            bv_sb = None
            if use_vbias:
                bv_sb = singles.tile([RC, 1], f32, tag="bv")
                nc.scalar.dma_start(bv_sb, bv)

            qT, kT, vb, aT = {}, {}, {}, {}
            last_dve = None
            for b in range(B):
                qT[b] = singles.tile([128, Tb], f32r, tag=f"qT{b}", name=f"qT{b}")
                kT[b] = singles.tile([128, Tb], f32r, tag=f"kT{b}", name=f"kT{b}")
                vb[b] = singles.tile([128, mt_per_b, 130], f32r, tag=f"vb{b}",
                                     name=f"vb{b}")
                aT[b] = singles.tile([128, Tb], f32r, tag=f"aT{b}", name=f"aT{b}")
                # ones columns for the softmax-denominator rows of PV
                nc.vector.memset(vb[b][:, :, 64:65].bitcast(f32), 1.0)
                last_dve = nc.vector.memset(vb[b][:, :, 129:130].bitcast(f32), 1.0)

            qkv_end_dve = {}

            def emit_qkv(b):
                nonlocal last_dve
                for mci in range(mc_per_b):
                    mc = b * mc_per_b + mci
                    x_sb = x_tiles[mc]
                    # qT / kT / vT: out[qkvcol, m] accumulated over 8 k-tiles
                    vTs = None
                    for nt in range(3):
                        ps = ps_a.tile([128, 512], f32, tag="mm")
                        for kt in range(8):
                            nc.tensor.matmul(
                                ps,
                                w3_sb[:, kt, nt * RC:(nt + 1) * RC],
                                x_sb[:, kt, :],
                                start=(kt == 0), stop=(kt == 7),
                            )
                        if nt < 2:
                            dest = qT[b] if nt == 0 else kT[b]
                            last_dve = nc.vector.tensor_scalar_add(
                                dest[:, mci * 512:(mci + 1) * 512], ps,
                                bqk_sb[:, nt:nt + 1],
                            )
                        else:
                            vTs = vtp.tile([128, 512], f32r, tag="vT")
                            last_dve = nc.vector.tensor_copy(vTs, ps)
                    # transpose vT chunks into natural [tokens, vcol] layout
                    tp = ps_a.tile([128, 512], f32, tag="mm")
                    for ms in range(4):
                        nc.tensor.transpose(
                            tp[:, ms * 128:(ms + 1) * 128].bitcast(f32r),
                            vTs[:, ms * 128:(ms + 1) * 128],
                            id_sb,
                        )
                    for ms in range(4):
                        mt = mci * 4 + ms
                        sl = tp[:, ms * 128:(ms + 1) * 128].bitcast(f32r)
                        nc.vector.tensor_copy(vb[b][:, mt, 0:64], sl[:, 0:64])
                        last_dve = nc.vector.tensor_copy(
                            vb[b][:, mt, 65:129], sl[:, 64:128]
                        )
                qkv_end_dve[b] = last_dve

            def emit_attn(b):
                nonlocal last_dve
                for qc in range(n_qc):
                    nkt = (qc + 1) * 4
                    pvs = (
                        ps_pv.tile([65, 512], f32, tag="pv", name="pv0"),
                        ps_pv.tile([65, 512], f32, tag="pv", name="pv1"),
                    )
                    # k-tiles in groups of 2: one [128, 2, 1024] psum
                    # S-block (4 banks, single-buffered), one exp per group
                    # (amortizes ACT per-instruction overhead); the PE fills
                    # the exp wait with the group's PV matmuls
                    for g in range(nkt // 2):
                        s = ps_s.tile([128, 2, 1024], f32, tag="s")
                        for j in range(2):
                            kt = g * 2 + j
                            for h in (0, 1):
                                nc.tensor.matmul(
                                    s[:, j, h * 512:(h + 1) * 512],
                                    kT[b][h * 64:(h + 1) * 64,
                                          kt * 128:(kt + 1) * 128],
                                    qT[b][h * 64:(h + 1) * 64,
                                          qc * 512:(qc + 1) * 512],
                                    start=True, stop=True,
                                )
                        pt = ptp.tile([128, 2, 1024], f32r, tag="pt")
                        nc.scalar.activation(pt, s, EXP, scale=0.125)
                        if g >= nkt // 2 - 2:  # diagonal groups: causal mask
                            pt2 = pt2p.tile([128, 2, 1024], f32r, tag="pt2")
                            for j in range(2):
                                kt = g * 2 + j
                                for h in (0, 1):
                                    nc.gpsimd.affine_select(
                                        pt2[:, j, h * 512:(h + 1) * 512],
                                        pt[:, j, h * 512:(h + 1) * 512],
                                        pattern=[[1, 512]],
                                        compare_op=IS_GE,
                                        fill=0.0,
                                        base=qc * 512 - kt * 128,
                                        channel_multiplier=-1,
                                    )
                            psrc = pt2
                        else:
                            psrc = pt
                        for j in range(2):
                            kt = g * 2 + j
                            for h in (0, 1):
                                nc.tensor.matmul(
                                    pvs[h],
                                    vb[b][:, kt, h * 65:(h + 1) * 65],
                                    psrc[:, j, h * 512:(h + 1) * 512],
                                    start=(kt == 0), stop=(kt == nkt - 1),
                                )
                    # batch both heads' denominators into one [2, 512]
                    # reciprocal and one DRAM bounce write; PV psum slots are
                    # freed by the SBUF copies, the rest runs off-path
                    slot = b * n_qc + qc
                    d1s = []
                    for h in (0, 1):
                        rc_t = rcp.tile([1, 512], f32, tag="rc", name="rc_t")
                        nc.vector.reciprocal(rc_t, pvs[h][64:65, :])
                        d1s.append(nc.sync.dma_start(scr_d[slot, h:h + 1, :], rc_t))
                    pvcs = []
                    for h in (0, 1):
                        pvc = pvcp.tile([65, 512], f32, tag="pvc")
                        if b == 0:
                            nc.vector.tensor_copy(pvc, pvs[h])
                        else:
                            nc.scalar.copy(pvc, pvs[h])
                        pvcs.append(pvc)
                    for h in (0, 1):
                        a0 = scr_d[slot, h:h + 1, :]
                        rb_t = rbp.tile([64, 512], f32, tag="rb")
                        d2 = nc.gpsimd.dma_start(rb_t, bass.AP(
                            tensor=a0.tensor, offset=a0.offset,
                            ap=[[0, 64], [1, 512]]))
                        add_dep_helper(d2.ins, d1s[h].ins, reason="scr bounce RAW")
                        dst = aT[b][h * 64:(h + 1) * 64, qc * 512:(qc + 1) * 512]
                        last_dve = nc.vector.tensor_tensor(
                            dst, pvcs[h][0:64, :], rb_t, op=MULT
                        )
                        if use_vbias:
                            last_dve = nc.vector.tensor_scalar_add(
                                dst, dst, bv_sb[h * 64:(h + 1) * 64, 0:1]
                            )
                    # projection for this q-chunk (pipelines into attention)
                    for nt in range(8):
                        ps = ps_a.tile([128, 512], f32, tag="mm")
                        nc.tensor.matmul(
                            ps,
                            wp_sb[:, nt * 128:(nt + 1) * 128],
                            aT[b][:, qc * 512:(qc + 1) * 512],
                            start=True, stop=True,
                        )
                        ob = obp.tile([128, 512], f32, tag="ob")
                        if b == 0:
                            last_dve = nc.vector.tensor_copy(ob, ps)
                        else:
                            nc.scalar.copy(ob, ps)
                        nc.sync.dma_start(
                            out_d[nt * 128:(nt + 1) * 128,
                                  b * Tb + qc * 512: b * Tb + (qc + 1) * 512],
                            ob,
                        )

            emit_qkv(0)
            emit_qkv(1)
            emit_attn(0)
            emit_attn(1)

    nc.compile()
    return nc


def make_in_maps(x, w_qkv, b_qkv, use_vbias):
    """Host-side shard prep. Returns per-core input maps (w_proj added later)."""
    Mx = x.shape[0] * x.shape[1]
    xT = np.ascontiguousarray(x.reshape(Mx, D_MODEL).T)
    in_maps = []
    for c in range(N_CORES):
        w3c = np.ascontiguousarray(
            np.concatenate(
                [w_qkv[:, s * D_MODEL + c * RC: s * D_MODEL + (c + 1) * RC]
                 for s in range(3)],
                axis=1,
            )
        )
        bqkc = np.ascontiguousarray(
            np.stack(
                [b_qkv[c * RC:(c + 1) * RC],
                 b_qkv[D_MODEL + c * RC: D_MODEL + (c + 1) * RC]],
                axis=1,
            )
        )
        im = {"xT": xT, "w3": w3c, "bqk": bqkc,
              "ident": np.eye(128, dtype=np.float32)}
        if use_vbias:
            im["bv"] = np.ascontiguousarray(
                b_qkv[2 * D_MODEL + c * RC: 2 * D_MODEL + (c + 1) * RC][:, None]
            )
        in_maps.append(im)
    return in_maps


def kernel(x, w_qkv, b_qkv, w_proj, b_proj):
    from concourse.bass_utils import run_bass_kernel_spmd

    x = np.asarray(x, dtype=np.float32)
    w_qkv = np.asarray(w_qkv, dtype=np.float32)
    b_qkv = np.asarray(b_qkv, dtype=np.float32)
    w_proj = np.asarray(w_proj, dtype=np.float32)
    b_proj = np.asarray(b_proj, dtype=np.float32)

    use_vbias = bool(np.any(b_qkv[2 * D_MODEL:]))
    key = (T, use_vbias)
    if key not in _prog_cache:
        _prog_cache[key] = build_program(T, use_vbias)
    nc = _prog_cache[key]

    in_maps = make_in_maps(x, w_qkv, b_qkv, use_vbias)
    for c in range(N_CORES):
        in_maps[c]["wp"] = np.ascontiguousarray(w_proj[c * RC:(c + 1) * RC, :])

    res = run_bass_kernel_spmd(nc, in_maps, core_ids=list(range(N_CORES)))
    global _last_results
    _last_results = res
    total = res.results[0]["out"].copy()
    for c in range(1, N_CORES):
        total += res.results[c]["out"]
    out = total.T.reshape(B, T, D_MODEL) + b_proj[None, None, :]
    return np.ascontiguousarray(out.astype(np.float32))



# revision 14
# speedup vs baseline: 1.5672x; 1.5672x over previous
"""Causal self-attention (B=2, T=2048, D=1024, H=16, Dh=64) on 8 NeuronCores.

Sharding: tensor-parallel over heads. Core c owns heads {2c, 2c+1}:
  - QKV: computes q/k/v columns c*128:(c+1)*128 of each section.
      q,k are produced transposed (qT/kT: [128 qkv-cols, tokens]) via
      out = w3_slice.T @ x.T matmuls; v is produced in natural layout
      ([tokens, 128 v-cols]) via PE transposes of the vT chunks.
  - Attention: per (batch, q-chunk of 512, k-tile of 128):
      S^T = K_h @ Q_h.T from kT/qT; the two heads' S matmuls sit in
      disjoint 64-row groups of the PE array and run concurrently
      (row-tiled 64x128 mode).  exp on ACT (no max subtraction needed:
      |S*scale| <= ~6), causal mask via in-place affine_select on the
      diagonal tiles (fill=0 after exp), then out^T accumulated as
      V'.T @ P^T where V' = [V | ones]: row 64 of the PSUM accumulator
      is the softmax denominator.
  - Projection: partial out^T = w_proj_slice.T applied per 128-row slice;
      per-core partial [1024, 4096] outputs are summed on the host.

Schedule notes (v2):
  - S psum is double-buffered per single k-tile ([128,1024] x 2) so the
    PE never waits for the exp of the previous tile.
  - The softmax normalize chain (scalar-copy psum->sbuf, DVE
    reciprocal_approx_fast, DRAM-bounce partition broadcast, gpsimd
    multiply) is fully off the PE path; the projection for q-chunk qc is
    emitted after the S/PV matmuls of qc+1 so its aT input is ready by
    the time the PE reaches it.
  - exp is trimmed on diagonal k-tiles (the fully-masked q range is
    skipped; affine_select fills it with 0 from stale data).
  - x is staged host-side into a chunk-major layout so each x DMA uses
    16KB contiguous descriptors, alternating between the two HWDGE
    queues (sync/scalar); outputs are staged into [128,1024] tiles (4KB
    descriptors) on the scalar queue.

All matmuls run in float32r (4-byte data, reduced-precision multiply,
1 cycle/row for moving dims >= 256).

TRN2 allows at most one sync-wait per instruction; bacc's
generate_event_semaphores pass splits multi-wait instructions, which is
why the program is built with bacc.Bacc and compiled before dispatch.
"""

import numpy as np

D_MODEL = 1024
B, T = 2, 2048
RC = 128  # per-core qkv columns per q/k/v section == per-core w_proj rows
M = B * T
N_CORES = 8

_prog_cache = {}
_last_results = None  # BassKernelResults of the most recent run (for profiling)


import os

INPLACE_MASK = os.environ.get("K_INPLACE_MASK", "1") == "1"
EXP_TRIM = os.environ.get("K_EXP_TRIM", "0") == "1"
# normalize modes: divide_v (DVE divide), divide_g (gpsimd divide),
# approx_full (custom-DVE reciprocal on full tile + gpsimd mult),
# recip (native reciprocal + gpsimd mult)
NORM_MODE = os.environ.get("K_NORM_MODE", "divide_v")


def build_program(Tb=T, use_vbias=False, use_qkbias=False):
    from contextlib import ExitStack

    import concourse.bass as bass
    import concourse.tile as tile
    from concourse import bacc, mybir
    from concourse.tile import add_dep_helper

    f32 = mybir.dt.float32
    f32r = mybir.dt.float32r
    EXP = mybir.ActivationFunctionType.Exp
    MULT = mybir.AluOpType.mult
    DIV = mybir.AluOpType.divide
    IS_GE = mybir.AluOpType.is_ge

    Mb = B * Tb
    mc_per_b = Tb // 512  # x/m chunks of 512 tokens per batch
    n_mc = B * mc_per_b
    n_qc = Tb // 512      # query chunks per batch

    nc = bacc.Bacc("TRN2", target_bir_lowering=False, debug=False)
    # x, chunk-major: [chunk, partition, ktile, token] -> 16KB descriptors
    xc = nc.dram_tensor("xc", [n_mc, 128, 8, 512], f32r, kind="ExternalInput").ap()
    w3 = nc.dram_tensor("w3", [D_MODEL, 3 * RC], f32r, kind="ExternalInput").ap()
    wp = nc.dram_tensor("wp", [RC, D_MODEL], f32r, kind="ExternalInput").ap()
    ident = nc.dram_tensor("ident", [128, 128], f32r, kind="ExternalInput").ap()
    bqk_d = bv_d = None
    if use_qkbias:
        bqk_d = nc.dram_tensor("bqk", [RC, 2], f32, kind="ExternalInput").ap()
    if use_vbias:
        bv_d = nc.dram_tensor("bv", [RC, 1], f32, kind="ExternalInput").ap()
    out_d = nc.dram_tensor("out", [D_MODEL, Mb], f32, kind="ExternalOutput").ap()
    scr_d = nc.dram_tensor("scr", [B * n_qc, 2, 512], f32).ap()  # recip bounce

    w3_r = w3.rearrange("(kt p) n -> p kt n", p=128)  # [128, 8, 384]

    with tile.TileContext(nc) as tc:
        with ExitStack() as ctx:
            singles = ctx.enter_context(tc.tile_pool(name="singles", bufs=1))
            xpool = ctx.enter_context(tc.tile_pool(name="xpool", bufs=2))
            ptp = ctx.enter_context(tc.tile_pool(name="ptp", bufs=3))
            pt2p = None
            if not INPLACE_MASK:
                pt2p = ctx.enter_context(tc.tile_pool(name="pt2p", bufs=2))
            rcp = ctx.enter_context(tc.tile_pool(name="rcp", bufs=2))
            rbp = ctx.enter_context(tc.tile_pool(name="rbp", bufs=3))
            vtp = ctx.enter_context(tc.tile_pool(name="vtp", bufs=2))
            pvcp = ctx.enter_context(tc.tile_pool(name="pvcp", bufs=3))
            obp = ctx.enter_context(tc.tile_pool(name="obp", bufs=12))
            ps_a = ctx.enter_context(tc.tile_pool(name="ps_a", bufs=2, space="PSUM"))
            ps_s = ctx.enter_context(tc.tile_pool(name="ps_s", bufs=2, space="PSUM"))
            ps_pv = ctx.enter_context(tc.tile_pool(name="ps_pv", bufs=2, space="PSUM"))

            # identity first (tiny), then PE warmup matmuls so the HAM clock
            # gate is released by the time the first x chunk lands
            id_sb = singles.tile([128, 128], f32r, tag="ident")
            nc.sync.dma_start(id_sb, ident)
            wu_ps = ps_a.tile([128, 512], f32, tag="mm")
            for _ in range(48):
                nc.tensor.matmul(wu_ps[:, 0:128], id_sb, id_sb,
                                 start=True, stop=True)

            # weights on the ACT HWDGE ring (before any output stores);
            # x chunks alternate between the SP and ACT rings
            w3_sb = singles.tile([128, 8, 3 * RC], f32r, tag="w3")
            nc.scalar.dma_start(w3_sb, w3_r)
            wp_sb = singles.tile([128, D_MODEL], f32r, tag="wp")
            nc.scalar.dma_start(wp_sb, wp)
            bqk_sb = bv_sb = None
            if use_qkbias:
                bqk_sb = singles.tile([RC, 2], f32, tag="bqk")
                nc.scalar.dma_start(bqk_sb, bqk_d)
            if use_vbias:
                bv_sb = singles.tile([RC, 1], f32, tag="bv")
                nc.scalar.dma_start(bv_sb, bv_d)

            x_tiles = []
            for mc in range(n_mc):
                x_sb = xpool.tile([128, 8, 512], f32r, tag="x")
                eng = nc.sync if mc % 2 == 0 else nc.scalar
                eng.dma_start(x_sb, xc[mc])
                x_tiles.append(x_sb)

            qT, kT, vb, aT = {}, {}, {}, {}
            for b in range(B):
                qT[b] = singles.tile([128, Tb], f32r, tag=f"qT{b}", name=f"qT{b}")
                kT[b] = singles.tile([128, Tb], f32r, tag=f"kT{b}", name=f"kT{b}")
                vb[b] = singles.tile([128, mc_per_b * 4, 130], f32r, tag=f"vb{b}",
                                     name=f"vb{b}")
                aT[b] = singles.tile([128, Tb], f32r, tag=f"aT{b}", name=f"aT{b}")
                # ones columns for the softmax-denominator rows of PV
                nc.vector.memset(vb[b][:, :, 64:65].bitcast(f32), 1.0)
                nc.vector.memset(vb[b][:, :, 129:130].bitcast(f32), 1.0)

            def emit_qkv(b):
                for mci in range(mc_per_b):
                    mc = b * mc_per_b + mci
                    x_sb = x_tiles[mc]
                    vTs = None
                    for nt in range(3):
                        ps = ps_a.tile([128, 512], f32, tag="mm")
                        for kt in range(8):
                            nc.tensor.matmul(
                                ps,
                                w3_sb[:, kt, nt * RC:(nt + 1) * RC],
                                x_sb[:, kt, :],
                                start=(kt == 0), stop=(kt == 7),
                            )
                        if nt < 2:
                            dest = (qT[b] if nt == 0 else kT[b])[
                                :, mci * 512:(mci + 1) * 512]
                            if use_qkbias:
                                nc.vector.tensor_scalar_add(
                                    dest, ps, bqk_sb[:, nt:nt + 1])
                            else:
                                nc.scalar.copy(dest, ps)
                        else:
                            vTs = vtp.tile([128, 512], f32r, tag="vT")
                            nc.scalar.copy(vTs, ps)
                    # transpose vT chunks into natural [tokens, vcol] layout
                    tp = ps_a.tile([128, 512], f32, tag="mm")
                    for ms in range(4):
                        nc.tensor.transpose(
                            tp[:, ms * 128:(ms + 1) * 128].bitcast(f32r),
                            vTs[:, ms * 128:(ms + 1) * 128],
                            id_sb,
                        )
                    for ms in range(4):
                        mt = mci * 4 + ms
                        sl = tp[:, ms * 128:(ms + 1) * 128].bitcast(f32r)
                        nc.vector.tensor_copy(vb[b][:, mt, 0:64], sl[:, 0:64])
                        nc.vector.tensor_copy(vb[b][:, mt, 65:129], sl[:, 64:128])

            def emit_attn_qc(b, qc):
                nkt = (qc + 1) * 4
                pvs = (
                    ps_pv.tile([65, 512], f32, tag="pv", name="pv0"),
                    ps_pv.tile([65, 512], f32, tag="pv", name="pv1"),
                )
                for kt in range(nkt):
                    diag = kt >= qc * 4
                    i = kt - qc * 4 if diag else 0
                    trim_s = min(i * 128, 256)  # keep moving dim >= 256
                    trim_e = i * 128 if EXP_TRIM else 0
                    s = ps_s.tile([128, 1024], f32, tag="s")
                    for h in (0, 1):
                        nc.tensor.matmul(
                            s[:, h * 512 + trim_s:(h + 1) * 512],
                            kT[b][h * 64:(h + 1) * 64,
                                  kt * 128:(kt + 1) * 128],
                            qT[b][h * 64:(h + 1) * 64,
                                  qc * 512 + trim_s:(qc + 1) * 512],
                            start=True, stop=True,
                        )
                    pt = ptp.tile([128, 1024], f32r, tag="pt")
                    if trim_e:
                        for h in (0, 1):
                            nc.scalar.activation(
                                pt[:, h * 512 + trim_e:(h + 1) * 512],
                                s[:, h * 512 + trim_e:(h + 1) * 512],
                                EXP, scale=0.125)
                    else:
                        nc.scalar.activation(pt, s, EXP, scale=0.125)
                    if diag:
                        # causal mask; also overwrites the stale
                        # (never-exp'd) fully-masked q range with fill=0
                        pdst = pt if INPLACE_MASK else pt2p.tile(
                            [128, 1024], f32r, tag="pt2")
                        for h in (0, 1):
                            nc.gpsimd.affine_select(
                                pdst[:, h * 512:(h + 1) * 512],
                                pt[:, h * 512:(h + 1) * 512],
                                pattern=[[1, 512]],
                                compare_op=IS_GE,
                                fill=0.0,
                                base=qc * 512 - kt * 128,
                                channel_multiplier=-1,
                            )
                        psrc = pdst
                    else:
                        psrc = pt
                    for h in (0, 1):
                        nc.tensor.matmul(
                            pvs[h],
                            vb[b][:, kt, h * 65:(h + 1) * 65],
                            psrc[:, h * 512:(h + 1) * 512],
                            start=(kt == 0), stop=(kt == nkt - 1),
                        )
                # softmax normalize: psum evacuated fast (vector copies) so
                # the PV psum slots recycle; everything after runs off the
                # PE-critical path
                slot = b * n_qc + qc
                pvcs, rds = [], []
                for h in (0, 1):
                    pvc = pvcp.tile([65, 512], f32, tag="pvc")
                    nc.vector.tensor_copy(pvc, pvs[h])
                    pvcs.append(pvc)
                # stage 1: get the per-q multiplier (or raw denominator) row
                # into scr_d for the partition broadcast
                for h in (0, 1):
                    if NORM_MODE.startswith("divide"):
                        rds.append(nc.sync.dma_start(
                            scr_d[slot, h:h + 1, :], pvcs[h][64:65, :]))
                    elif NORM_MODE == "approx_full":
                        rc_t = rcp.tile([65, 512], f32, tag="rc", name="rc_t")
                        nc.vector.reciprocal_approx_fast(rc_t, pvcs[h])
                        rds.append(nc.sync.dma_start(
                            scr_d[slot, h:h + 1, :], rc_t[64:65, :]))
                    else:
                        rc_t = rcp.tile([1, 512], f32, tag="rc", name="rc_t")
                        nc.vector.reciprocal(rc_t, pvcs[h][64:65, :])
                        rds.append(nc.sync.dma_start(
                            scr_d[slot, h:h + 1, :], rc_t))
                # stage 2: broadcast-read and apply
                for h in (0, 1):
                    a0 = scr_d[slot, h:h + 1, :]
                    rb_t = rbp.tile([64, 512], f32, tag="rb")
                    d2 = nc.gpsimd.dma_start(rb_t, bass.AP(
                        tensor=a0.tensor, offset=a0.offset,
                        ap=[[0, 64], [1, 512]]))
                    add_dep_helper(d2.ins, rds[h].ins, reason="scr bounce RAW")
                    dst = aT[b][h * 64:(h + 1) * 64, qc * 512:(qc + 1) * 512]
                    if NORM_MODE == "divide_v":
                        nc.vector.tensor_tensor(
                            dst, pvcs[h][0:64, :], rb_t, op=DIV)
                    elif NORM_MODE == "divide_g":
                        nc.gpsimd.tensor_tensor(
                            dst, pvcs[h][0:64, :], rb_t, op=DIV)
                    else:
                        nc.gpsimd.tensor_tensor(
                            dst, pvcs[h][0:64, :], rb_t, op=MULT)
                    if use_vbias:
                        nc.gpsimd.tensor_scalar_add(
                            dst, dst, bv_sb[h * 64:(h + 1) * 64, 0:1])

            ob_tiles = {}

            def emit_proj(b, qc):
                qp, half = qc // 2, qc % 2
                if half == 0:
                    ob_tiles[(b, qp)] = [
                        obp.tile([128, 1024], f32, tag="ob", name=f"ob{nt}")
                        for nt in range(8)]
                obs = ob_tiles[(b, qp)]
                for nt in range(8):
                    ps = ps_a.tile([128, 512], f32, tag="mm")
                    nc.tensor.matmul(
                        ps,
                        wp_sb[:, nt * 128:(nt + 1) * 128],
                        aT[b][:, qc * 512:(qc + 1) * 512],
                        start=True, stop=True,
                    )
                    dst = obs[nt][:, half * 512:(half + 1) * 512]
                    nc.vector.tensor_copy(dst, ps)
                if half == 1:
                    # stores dispatch on the SP queue: the ACT queue would
                    # delay the next q-chunk's exp behind the store triggers
                    for nt in range(8):
                        nc.sync.dma_start(
                            out_d[nt * 128:(nt + 1) * 128,
                                  b * Tb + qp * 1024: b * Tb + (qp + 1) * 1024],
                            obs[nt],
                        )
                    del ob_tiles[(b, qp)]

            emit_qkv(0)
            emit_qkv(1)
            sched = [(b, qc) for b in range(B) for qc in range(n_qc)]
            prev = None
            for b, qc in sched:
                emit_attn_qc(b, qc)
                if prev is not None:
                    emit_proj(*prev)
                prev = (b, qc)
            emit_proj(*prev)

    nc.compile()
    return nc


def make_in_maps(x, w_qkv, b_qkv, use_vbias, use_qkbias):
    """Host-side shard prep. Returns per-core input maps (w_proj added later)."""
    Mx = x.shape[0] * x.shape[1]
    xT = np.ascontiguousarray(x.reshape(Mx, D_MODEL).T)  # [1024, 4096]
    # chunk-major: [chunk, partition, ktile, token-in-chunk]
    xcm = np.ascontiguousarray(
        xT.reshape(8, 128, Mx // 512, 512).transpose(2, 1, 0, 3)
    )
    in_maps = []
    for c in range(N_CORES):
        w3c = np.ascontiguousarray(
            np.concatenate(
                [w_qkv[:, s * D_MODEL + c * RC: s * D_MODEL + (c + 1) * RC]
                 for s in range(3)],
                axis=1,
            )
        )
        im = {"xc": xcm, "w3": w3c,
              "ident": np.eye(128, dtype=np.float32)}
        if use_qkbias:
            im["bqk"] = np.ascontiguousarray(
                np.stack(
                    [b_qkv[c * RC:(c + 1) * RC],
                     b_qkv[D_MODEL + c * RC: D_MODEL + (c + 1) * RC]],
                    axis=1,
                )
            )
        if use_vbias:
            im["bv"] = np.ascontiguousarray(
                b_qkv[2 * D_MODEL + c * RC: 2 * D_MODEL + (c + 1) * RC][:, None]
            )
        in_maps.append(im)
    return in_maps


def kernel(x, w_qkv, b_qkv, w_proj, b_proj):
    from concourse.bass_utils import run_bass_kernel_spmd

    x = np.asarray(x, dtype=np.float32)
    w_qkv = np.asarray(w_qkv, dtype=np.float32)
    b_qkv = np.asarray(b_qkv, dtype=np.float32)
    w_proj = np.asarray(w_proj, dtype=np.float32)
    b_proj = np.asarray(b_proj, dtype=np.float32)

    use_vbias = bool(np.any(b_qkv[2 * D_MODEL:]))
    use_qkbias = bool(np.any(b_qkv[:2 * D_MODEL]))
    key = (T, use_vbias, use_qkbias)
    if key not in _prog_cache:
        _prog_cache[key] = build_program(T, use_vbias, use_qkbias)
    nc = _prog_cache[key]

    in_maps = make_in_maps(x, w_qkv, b_qkv, use_vbias, use_qkbias)
    for c in range(N_CORES):
        in_maps[c]["wp"] = np.ascontiguousarray(w_proj[c * RC:(c + 1) * RC, :])

    res = run_bass_kernel_spmd(nc, in_maps, core_ids=list(range(N_CORES)))
    global _last_results
    _last_results = res
    total = res.results[0]["out"].copy()
    for c in range(1, N_CORES):
        total += res.results[c]["out"]
    out = total.T.reshape(B, T, D_MODEL) + b_proj[None, None, :]
    return np.ascontiguousarray(out.astype(np.float32))
